# revision 36
# baseline (speedup 1.0000x reference)
"""AlphaPermutationLayer Trainium2 kernel (v2).

out[i, j] = sum_k softmax(alpha/T)[k] * (perm[k, i] == j),  N=2048, K=64.

Sharding: output ROWS across the 8 cores (row i depends only on perm[:, i]
and alpha — no collective).  Per core (256 rows), digit-split
j = jq*64 + jf (jq in [0,32), jf in [0,64)); one matmul per row i:
    out_i[jq, jf] = sum_k A_i[k, jq] * B_i[k, jf]
with A = alpha-scaled onehot(perm>>6) stationary ([64, 32], LDW 32 cols)
and B = onehot(perm&63) moving ([64, 64]).  Rows are processed two-per-
partition-set: partition p = k + 64*h holds row half h, so DVE one-hot
builds use all 128 lanes while each matmul contracts 64 partitions at
tile_position (64h, 32g) — 4 col-groups give concurrent matmuls.  Single
bf16 pass (no hi/lo): alpha rounding ~2e-3 rel, gate is 2e-2.  PSUM holds
the whole 2MB per-core output; ACT evacuates with the fused 1/S softmax
normalization; strided DMAs (256B runs) write DRAM.
"""

import os
import sys

sys.path.insert(0, "/opt/trn_rl_repo")

import numpy as np

N = 2048
K = 64
NCORES = 8
ROWS = N // NCORES          # 256 rows per core
Q = 32                      # stationary digit width (jq)
F = 64                      # moving digit width (jf)
CW = 32                     # i2 chunk width (4 chunks of 32)
IW = 8                      # iota block width (replayed via 0-stride AP)

LAST_EXEC_NS = None
LAST_RESULTS = None

_cached = {}


def _build_bass():
    import concourse.tile as tile
    from concourse import bacc, mybir

    fp32 = mybir.dt.float32
    bf16 = mybir.dt.bfloat16
    i16 = mybir.dt.int16
    Copy = mybir.ActivationFunctionType.Copy
    Exp = mybir.ActivationFunctionType.Exp
    IsEq = mybir.AluOpType.is_equal

    nc = bacc.Bacc()

    phl_ext = nc.declare_dram_parameter("phl", [128, 256], i16, isOutput=False)
    at_ext = nc.declare_dram_parameter("altp", [128, 2], fp32, isOutput=False)
    io_ext = nc.declare_dram_parameter("iota", [(Q + F) * IW], i16, isOutput=False)
    out_ext = nc.declare_dram_parameter("out", [ROWS, N], fp32, isOutput=True)

    with tile.TileContext(nc) as tc:
        with (
            tc.tile_pool(name="sbuf", bufs=1) as sb,
            tc.tile_pool(name="stage", bufs=6) as stp,
            tc.tile_pool(name="smax_psum", bufs=1, space="PSUM") as psmax,
            tc.tile_pool(name="psum", bufs=7, space="PSUM") as pp,
        ):
            # ---- input loads: 3 DMAs total, perm digits first ---------------
            phl_t = sb.tile([128, 256], i16)
            at_t = sb.tile([128, 2], fp32)
            io_t = sb.tile([128, Q + F, IW], i16)  # [p, d, iu]: iq then if
            nc.sync.dma_start(out=phl_t[:], in_=phl_ext[:])
            nc.scalar.dma_start(
                out=io_t[:],
                in_=io_ext[:].rearrange("(d i) -> d i", i=IW).partition_broadcast(128),
            )
            nc.scalar.dma_start(out=at_t[:], in_=at_ext[:])
            ph_t = phl_t[:, 0:128]
            pl_t = phl_t[:, 128:256]
            iq_t = io_t[:, 0:Q]
            if_t = io_t[:, Q : Q + F]
            al_t = at_t[:, 0:1]
            tp_t = at_t[:, 1:2]

            # ---- softmax head ----------------------------------------------
            # e = exp(alpha/T) unnormalized; S recovered via matmul with 0.5
            # (partitions hold k twice), 1/S applied at evacuation.
            rt_t = sb.tile([128, 1], fp32)
            e_t = sb.tile([128, 1], fp32)
            ln2_t = sb.tile([128, 1], fp32)
            prime_t = sb.tile([128, 1], fp32)
            half_col = sb.tile([128, 1], fp32)
            ones_row = sb.tile([1, 128], fp32)
            r_t = sb.tile([1, 1], fp32)
            rs_t = sb.tile([128, 1], fp32)
            nc.vector.memset(ln2_t[:], float(np.log(2.0)))
            nc.vector.memset(half_col[:], 0.5)
            nc.vector.memset(ones_row[:], 1.0)
            # dep-free ACT op: hoists the one-time activation-table load off
            # the exp critical path.
            nc.scalar.activation(out=prime_t[:], in_=ln2_t[:], func=Exp)
            sum_ps = psmax.tile([1, 1], fp32, tag="smax")

            a_t = sb.tile([128, Q, 128], bf16)   # [p, jq, i2] pure one-hot
            a_s = sb.tile([128, Q, 128], bf16)   # alpha-scaled one-hot
            b_t = sb.tile([128, F, 128], bf16)   # [p, jf, i2] pure one-hot

            # DRAM view: row i = 32b + 4s + g, col j = q*64 + f;
            # psum partition = 32g + q, psum free = 64s + f.  With g the LOW
            # row bits, the DRAM dims (g, q) merge to one stride-64 dim, so
            # the whole bank drains in ONE 3-dim dma_start.
            oview = out_ext[:].rearrange(
                "(b s g) (q f) -> b g q s f", b=8, s=8, g=4, q=Q, f=F
            )

            banks = [None] * 8
            rb_ps = []

            NR = CW // IW    # 0-stride replays of the iota block per chunk

            def emit_chunk(c):
                ic = slice(CW * c, CW * c + CW)
                # 4D APs: chunk i2 = (ih, il) with il=IW; the iota block is
                # replayed across ih via a 0-stride dim (inner stays step-1
                # so the DVE keeps 2x mode).
                nc.vector.tensor_tensor(
                    out=a_t[:, :, ic].rearrange(
                        "p q (ih il) -> p q ih il", il=IW
                    ),
                    in0=ph_t[:, ic].rearrange("p (ih il) -> p ih il", il=IW)
                    .unsqueeze(1)
                    .to_broadcast([128, Q, NR, IW]),
                    in1=iq_t[:].unsqueeze(2).to_broadcast([128, Q, NR, IW]),
                    op=IsEq,
                )
                nc.vector.tensor_tensor(
                    out=b_t[:, :, ic].rearrange(
                        "p f (ih il) -> p f ih il", il=IW
                    ),
                    in0=pl_t[:, ic].rearrange("p (ih il) -> p ih il", il=IW)
                    .unsqueeze(1)
                    .to_broadcast([128, F, NR, IW]),
                    in1=if_t[:].unsqueeze(2).to_broadcast([128, F, NR, IW]),
                    op=IsEq,
                )
                if c == 0:
                    # alpha chain, placed AFTER the first builds so the DVE
                    # doesn't stall on the altp DMA before starting them.
                    nc.vector.reciprocal(out=rt_t[:], in_=tp_t[:])
                    nc.scalar.activation(
                        out=e_t[:], in_=al_t[:], func=Exp, scale=rt_t[:]
                    )
                    nc.tensor.matmul(
                        sum_ps[:], lhsT=e_t[:], rhs=half_col[:],
                        start=True, stop=True,
                    )
                nc.vector.tensor_scalar(
                    out=a_s[:, :, ic], in0=a_t[:, :, ic], scalar1=e_t[:],
                    scalar2=None, op0=mybir.AluOpType.mult,
                )
                if c == 0:
                    nc.vector.reciprocal(out=r_t[:], in_=sum_ps[:])
                if c == 1:
                    nc.vector.tensor_copy(out=rs_t[:], in_=rb_ps[0][:])
                for h in range(2):
                    banks[c + 4 * h] = pp.tile(
                        [128, 8, F], fp32, tag="bank", name=f"bank{c}_{h}"
                    )
                # h fastest, then g: consecutive matmuls alternate row-groups
                # (LDW overlap) and col-groups (stream concurrency).
                for s in range(8):
                    for g in range(4):
                        for h in range(2):
                            i2 = CW * c + 4 * s + g
                            kp = slice(64 * h, 64 * h + 64)
                            nc.tensor.matmul(
                                banks[c + 4 * h][32 * g : 32 * g + 32, s],
                                lhsT=a_s[kp, :, i2],
                                rhs=b_t[kp, :, i2],
                                start=True,
                                stop=True,
                                tile_position=(64 * h, 32 * g),
                            )
                if c == 0:
                    # 1/S broadcast to all partitions, between MM bursts.
                    rb = psmax.tile([128, 1], fp32, tag="smax", name="rb_ps")
                    rb_ps.append(rb)
                    nc.tensor.matmul(
                        rb[:], lhsT=ones_row[:], rhs=r_t[:],
                        start=True, stop=True,
                    )

            def emit_drain(c):
                for h in range(2):
                    bi = c + 4 * h
                    stage = stp.tile([128, 8, F], fp32, tag="stage")
                    nc.scalar.activation(
                        out=stage[:], in_=banks[bi][:], func=Copy,
                        scale=rs_t[:],
                    )
                    eng = nc.sync if bi % 2 == 0 else nc.scalar
                    eng.dma_start(out=oview[bi], in_=stage[:])

            for c in range(4):
                emit_chunk(c)
                if c >= 1:
                    emit_drain(c - 1)
            emit_drain(3)

    if not nc.is_finalized():
        nc.finalize()
    return nc


def _prep_inputs(alpha_weights, perm_vectors, temperature):
    a = np.asarray(alpha_weights, dtype=np.float32).reshape(K)
    T = np.asarray(temperature, dtype=np.float32).reshape(())
    perm = np.asarray(perm_vectors).astype(np.int64).reshape(K, N)
    ph = (perm >> 6).astype(np.int16)
    pl = (perm & 63).astype(np.int16)
    al_t = np.concatenate([a, a])[:, None].copy()          # [128, 1]
    tp_t = np.full((128, 1), T, dtype=np.float32)
    iota = np.concatenate(
        [np.repeat(np.arange(Q), IW), np.repeat(np.arange(F), IW)]
    ).astype(np.int16)
    in_maps = []
    for c in range(NCORES):
        # partition p = k + 64*h, column i2: row r = 128*h + i2 of this core
        phc = ph[:, c * ROWS : (c + 1) * ROWS].reshape(K, 2, 128)
        plc = pl[:, c * ROWS : (c + 1) * ROWS].reshape(K, 2, 128)
        phl = np.concatenate(
            [
                phc.transpose(1, 0, 2).reshape(128, 128),
                plc.transpose(1, 0, 2).reshape(128, 128),
            ],
            axis=1,
        ).copy()
        in_maps.append(
            {
                "phl": phl,
                "altp": np.concatenate([al_t, tp_t], axis=1).copy(),
                "iota": iota,
            }
        )
    return in_maps


def _install_ntff_hook():
    """Provide antenv.axon_hooks (missing in this image) so that
    run_bass_kernel_spmd(trace=True) can capture NTFF profiles via the
    axon PJRT .so (same mechanism as trn_agent_boot.trn_boot)."""
    import contextlib
    import ctypes
    import types

    try:
        from antenv.axon_hooks import get_axon_ntff_profile_hook  # noqa: F401

        return True
    except ImportError:
        pass
    so_path = "/opt/axon/libaxon_pjrt.so"
    if not os.path.exists(so_path):
        return False
    lib = ctypes.CDLL(so_path)
    if not hasattr(lib, "axon_start_nrt_profile"):
        return False
    lib.axon_start_nrt_profile.argtypes = [
        ctypes.POINTER(ctypes.c_int64),
        ctypes.c_size_t,
    ]
    lib.axon_start_nrt_profile.restype = ctypes.c_int64
    lib.axon_stop_nrt_profile.argtypes = [ctypes.c_char_p]
    lib.axon_stop_nrt_profile.restype = ctypes.c_int64

    @contextlib.contextmanager
    def _hook(output_dir, device_ids):
        import jax

        jax.devices()
        if device_ids:
            ids = (ctypes.c_int64 * len(device_ids))(*device_ids)
            rc = lib.axon_start_nrt_profile(ids, len(device_ids))
        else:
            rc = lib.axon_start_nrt_profile(None, 0)
        if rc != 0:
            raise RuntimeError(f"axon_start_nrt_profile rc={rc}")
        try:
            yield
        finally:
            n = lib.axon_stop_nrt_profile(str(output_dir).encode())
            print(f"ntff profile: {n} file(s) written to {output_dir}")

    import antenv

    mod = types.ModuleType("antenv.axon_hooks")
    mod.get_axon_ntff_profile_hook = lambda: _hook
    mod.set_axon_ntff_profile_hook = lambda h: None
    sys.modules["antenv.axon_hooks"] = mod
    antenv.axon_hooks = mod
    return True


def kernel(alpha_weights, perm_vectors, temperature):
    global LAST_EXEC_NS, LAST_RESULTS
    from concourse.bass_utils import run_bass_kernel_spmd

    if "nc" not in _cached:
        _cached["nc"] = _build_bass()
    nc = _cached["nc"]
    in_maps = _prep_inputs(alpha_weights, perm_vectors, temperature)
    core_ids = list(range(NCORES))
    trace = os.environ.get("KERNEL_TRACE", "0") == "1"
    if trace:
        trace = _install_ntff_hook()
    try:
        res = run_bass_kernel_spmd(nc, in_maps, core_ids, trace=trace)
    except Exception:
        if not trace:
            raise
        res = run_bass_kernel_spmd(nc, in_maps, core_ids, trace=False)
    LAST_EXEC_NS = res.exec_time_ns
    LAST_RESULTS = res
    out = np.concatenate([res.results[c]["out"] for c in range(NCORES)], axis=0)
    return out.astype(np.float32)


if __name__ == "__main__":
    rng = np.random.default_rng(0)
    a = rng.standard_normal(K).astype(np.float32)
    perm = np.stack([rng.permutation(N) for _ in range(K)]).astype(np.int64)
    T = np.ones((), np.float32)
    out = kernel(a, perm, T)
    # numpy reference
    al = np.exp(a / T - (a / T).max())
    al /= al.sum()
    exp = np.zeros((N, N), np.float32)
    np.add.at(exp, (np.broadcast_to(np.arange(N), (K, N)), perm), al[:, None])
    print("max abs err:", np.abs(out - exp).max(), "max ref:", np.abs(exp).max())
    print("exec ns:", LAST_EXEC_NS)


# revision 42
# speedup vs baseline: 1.0659x; 1.0659x over previous
"""AlphaPermutationLayer Trainium2 kernel (v2).

out[i, j] = sum_k softmax(alpha/T)[k] * (perm[k, i] == j),  N=2048, K=64.

Sharding: output ROWS across the 8 cores (row i depends only on perm[:, i]
and alpha — no collective).  Per core (256 rows), digit-split
j = jq*64 + jf (jq in [0,32), jf in [0,64)); one matmul per row i:
    out_i[jq, jf] = sum_k A_i[k, jq] * B_i[k, jf]
with A = alpha-scaled onehot(perm>>6) stationary ([64, 32], LDW 32 cols)
and B = onehot(perm&63) moving ([64, 64]).  Rows are processed two-per-
partition-set: partition p = k + 64*h holds row half h, so DVE one-hot
builds use all 128 lanes while each matmul contracts 64 partitions at
tile_position (64h, 32g) — 4 col-groups give concurrent matmuls.  Single
bf16 pass (no hi/lo): alpha rounding ~2e-3 rel, gate is 2e-2.  PSUM holds
the whole 2MB per-core output; ACT evacuates with the fused 1/S softmax
normalization; strided DMAs (256B runs) write DRAM.
"""

import os
import sys

sys.path.insert(0, "/opt/trn_rl_repo")

import numpy as np

N = 2048
K = 64
NCORES = 8
ROWS = N // NCORES          # 256 rows per core
Q = 32                      # stationary digit width (jq)
F = 64                      # moving digit width (jf)
CW = 32                     # i2 chunk width (4 chunks of 32)
IW = 8                      # iota block width (replayed via 0-stride AP)

LAST_EXEC_NS = None
LAST_RESULTS = None

_cached = {}


def _build_bass():
    import concourse.tile as tile
    from concourse import bacc, mybir

    fp32 = mybir.dt.float32
    bf16 = mybir.dt.bfloat16
    i16 = mybir.dt.int16
    Copy = mybir.ActivationFunctionType.Copy
    Exp = mybir.ActivationFunctionType.Exp
    IsEq = mybir.AluOpType.is_equal

    nc = bacc.Bacc()

    phl_ext = nc.declare_dram_parameter("phl", [128, 256], i16, isOutput=False)
    at_ext = nc.declare_dram_parameter("altp", [128, 2], fp32, isOutput=False)
    out_ext = nc.declare_dram_parameter("out", [ROWS, N], fp32, isOutput=True)

    with tile.TileContext(nc) as tc:
        with (
            tc.tile_pool(name="sbuf", bufs=1) as sb,
            tc.tile_pool(name="stage", bufs=6) as stp,
            tc.tile_pool(name="smax_psum", bufs=1, space="PSUM") as psmax,
            tc.tile_pool(name="psum", bufs=7, space="PSUM") as pp,
        ):
            # ---- input loads: 2 DMAs total; iotas generated on-chip ---------
            phl_t = sb.tile([128, 256], i16)
            at_t = sb.tile([128, 2], fp32)
            io_t = sb.tile([128, Q + F, IW], i16)  # [p, d, iu]: iq then if
            nc.sync.dma_start(out=phl_t[:], in_=phl_ext[:])
            nc.scalar.dma_start(out=at_t[:], in_=at_ext[:])
            iq_t = io_t[:, 0:Q]
            if_t = io_t[:, Q : Q + F]
            nc.gpsimd.iota(iq_t, pattern=[[1, Q], [0, IW]], channel_multiplier=0)
            nc.gpsimd.iota(if_t, pattern=[[1, F], [0, IW]], channel_multiplier=0)
            ph_t = phl_t[:, 0:128]
            pl_t = phl_t[:, 128:256]
            al_t = at_t[:, 0:1]
            tp_t = at_t[:, 1:2]

            # ---- softmax head ----------------------------------------------
            # e = exp(alpha/T) unnormalized; S recovered via matmul with 0.5
            # (partitions hold k twice), 1/S applied at evacuation.
            rt_t = sb.tile([128, 1], fp32)
            e_t = sb.tile([128, 1], fp32)
            ln2_t = sb.tile([128, 1], fp32)
            prime_t = sb.tile([128, 1], fp32)
            half_col = sb.tile([128, 1], fp32)
            ones_row = sb.tile([1, 128], fp32)
            r_t = sb.tile([1, 1], fp32)
            rs_t = sb.tile([128, 1], fp32)
            nc.vector.memset(ln2_t[:], float(np.log(2.0)))
            nc.vector.memset(half_col[:], 0.5)
            nc.vector.memset(ones_row[:], 1.0)
            # dep-free ACT op: hoists the one-time activation-table load off
            # the exp critical path.
            nc.scalar.activation(out=prime_t[:], in_=ln2_t[:], func=Exp)
            sum_ps = psmax.tile([1, 1], fp32, tag="smax")

            a_t = sb.tile([128, Q, 128], bf16)   # [p, jq, i2] pure one-hot
            a_s = sb.tile([128, Q, 128], bf16)   # alpha-scaled one-hot
            b_t = sb.tile([128, F, 128], bf16)   # [p, jf, i2] pure one-hot

            # DRAM view: row i = 32b + 4s + g, col j = q*64 + f;
            # psum partition = 32g + q, psum free = 64s + f.  With g the LOW
            # row bits, the DRAM dims (g, q) merge to one stride-64 dim, so
            # the whole bank drains in ONE 3-dim dma_start.
            oview = out_ext[:].rearrange(
                "(b s g) (q f) -> b g q s f", b=8, s=8, g=4, q=Q, f=F
            )

            banks = [None] * 8
            rb_ps = []

            NR = CW // IW    # 0-stride replays of the iota block per chunk

            def emit_chunk(c):
                ic = slice(CW * c, CW * c + CW)
                # 4D APs: chunk i2 = (ih, il) with il=IW; the iota block is
                # replayed across ih via a 0-stride dim (inner stays step-1
                # so the DVE keeps 2x mode).
                nc.vector.tensor_tensor(
                    out=a_t[:, :, ic].rearrange(
                        "p q (ih il) -> p q ih il", il=IW
                    ),
                    in0=ph_t[:, ic].rearrange("p (ih il) -> p ih il", il=IW)
                    .unsqueeze(1)
                    .to_broadcast([128, Q, NR, IW]),
                    in1=iq_t[:].unsqueeze(2).to_broadcast([128, Q, NR, IW]),
                    op=IsEq,
                )
                nc.vector.tensor_tensor(
                    out=b_t[:, :, ic].rearrange(
                        "p f (ih il) -> p f ih il", il=IW
                    ),
                    in0=pl_t[:, ic].rearrange("p (ih il) -> p ih il", il=IW)
                    .unsqueeze(1)
                    .to_broadcast([128, F, NR, IW]),
                    in1=if_t[:].unsqueeze(2).to_broadcast([128, F, NR, IW]),
                    op=IsEq,
                )
                if c == 0:
                    # alpha chain, placed AFTER the first builds so the DVE
                    # doesn't stall on the altp DMA before starting them.
                    nc.vector.reciprocal(out=rt_t[:], in_=tp_t[:])
                    nc.scalar.activation(
                        out=e_t[:], in_=al_t[:], func=Exp, scale=rt_t[:]
                    )
                    nc.tensor.matmul(
                        sum_ps[:], lhsT=e_t[:], rhs=half_col[:],
                        start=True, stop=True,
                    )
                nc.vector.tensor_scalar(
                    out=a_s[:, :, ic], in0=a_t[:, :, ic], scalar1=e_t[:],
                    scalar2=None, op0=mybir.AluOpType.mult,
                )
                if c == 0:
                    nc.vector.reciprocal(out=r_t[:], in_=sum_ps[:])
                if c == 1:
                    nc.vector.tensor_copy(out=rs_t[:], in_=rb_ps[0][:])
                for h in range(2):
                    banks[c + 4 * h] = pp.tile(
                        [128, 8, F], fp32, tag="bank", name=f"bank{c}_{h}"
                    )
                # h OUTER: consecutive matmuls stay in one row-group, so the
                # PE runs a single serialized stream (g-alternation only
                # overlaps LDW).  Concurrent h-interleaved streams are faster
                # on paper but saturate SBUF read bw and throttle DVE/ACT
                # ops 2-3x (measured), a net loss.
                for h in range(2):
                    for s in range(8):
                        for g in range(4):
                            i2 = CW * c + 4 * s + g
                            kp = slice(64 * h, 64 * h + 64)
                            nc.tensor.matmul(
                                banks[c + 4 * h][32 * g : 32 * g + 32, s],
                                lhsT=a_s[kp, :, i2],
                                rhs=b_t[kp, :, i2],
                                start=True,
                                stop=True,
                                tile_position=(64 * h, 32 * g),
                            )
                if c == 0:
                    # 1/S broadcast to all partitions, between MM bursts.
                    rb = psmax.tile([128, 1], fp32, tag="smax", name="rb_ps")
                    rb_ps.append(rb)
                    nc.tensor.matmul(
                        rb[:], lhsT=ones_row[:], rhs=r_t[:],
                        start=True, stop=True,
                    )

            def emit_drain(c):
                for h in range(2):
                    bi = c + 4 * h
                    stage = stp.tile([128, 8, F], fp32, tag="stage")
                    nc.scalar.activation(
                        out=stage[:], in_=banks[bi][:], func=Copy,
                        scale=rs_t[:],
                    )
                    eng = nc.sync if bi % 2 == 0 else nc.scalar
                    eng.dma_start(out=oview[bi], in_=stage[:])

            for c in range(4):
                emit_chunk(c)
                if c >= 1:
                    emit_drain(c - 1)
            emit_drain(3)

    if not nc.is_finalized():
        nc.finalize()
    return nc


def _prep_inputs(alpha_weights, perm_vectors, temperature):
    a = np.asarray(alpha_weights, dtype=np.float32).reshape(K)
    T = np.asarray(temperature, dtype=np.float32).reshape(())
    perm = np.asarray(perm_vectors).astype(np.int64).reshape(K, N)
    ph = (perm >> 6).astype(np.int16)
    pl = (perm & 63).astype(np.int16)
    al_t = np.concatenate([a, a])[:, None].copy()          # [128, 1]
    tp_t = np.full((128, 1), T, dtype=np.float32)
    in_maps = []
    for c in range(NCORES):
        # partition p = k + 64*h, column i2: row r = 128*h + i2 of this core
        phc = ph[:, c * ROWS : (c + 1) * ROWS].reshape(K, 2, 128)
        plc = pl[:, c * ROWS : (c + 1) * ROWS].reshape(K, 2, 128)
        phl = np.concatenate(
            [
                phc.transpose(1, 0, 2).reshape(128, 128),
                plc.transpose(1, 0, 2).reshape(128, 128),
            ],
            axis=1,
        ).copy()
        in_maps.append(
            {
                "phl": phl,
                "altp": np.concatenate([al_t, tp_t], axis=1).copy(),
            }
        )
    return in_maps


def _install_ntff_hook():
    """Provide antenv.axon_hooks (missing in this image) so that
    run_bass_kernel_spmd(trace=True) can capture NTFF profiles via the
    axon PJRT .so (same mechanism as trn_agent_boot.trn_boot)."""
    import contextlib
    import ctypes
    import types

    try:
        from antenv.axon_hooks import get_axon_ntff_profile_hook  # noqa: F401

        return True
    except ImportError:
        pass
    so_path = "/opt/axon/libaxon_pjrt.so"
    if not os.path.exists(so_path):
        return False
    lib = ctypes.CDLL(so_path)
    if not hasattr(lib, "axon_start_nrt_profile"):
        return False
    lib.axon_start_nrt_profile.argtypes = [
        ctypes.POINTER(ctypes.c_int64),
        ctypes.c_size_t,
    ]
    lib.axon_start_nrt_profile.restype = ctypes.c_int64
    lib.axon_stop_nrt_profile.argtypes = [ctypes.c_char_p]
    lib.axon_stop_nrt_profile.restype = ctypes.c_int64

    @contextlib.contextmanager
    def _hook(output_dir, device_ids):
        import jax

        jax.devices()
        if device_ids:
            ids = (ctypes.c_int64 * len(device_ids))(*device_ids)
            rc = lib.axon_start_nrt_profile(ids, len(device_ids))
        else:
            rc = lib.axon_start_nrt_profile(None, 0)
        if rc != 0:
            raise RuntimeError(f"axon_start_nrt_profile rc={rc}")
        try:
            yield
        finally:
            n = lib.axon_stop_nrt_profile(str(output_dir).encode())
            print(f"ntff profile: {n} file(s) written to {output_dir}")

    import antenv

    mod = types.ModuleType("antenv.axon_hooks")
    mod.get_axon_ntff_profile_hook = lambda: _hook
    mod.set_axon_ntff_profile_hook = lambda h: None
    sys.modules["antenv.axon_hooks"] = mod
    antenv.axon_hooks = mod
    return True


def kernel(alpha_weights, perm_vectors, temperature):
    global LAST_EXEC_NS, LAST_RESULTS
    from concourse.bass_utils import run_bass_kernel_spmd

    if "nc" not in _cached:
        _cached["nc"] = _build_bass()
    nc = _cached["nc"]
    in_maps = _prep_inputs(alpha_weights, perm_vectors, temperature)
    core_ids = list(range(NCORES))
    trace = os.environ.get("KERNEL_TRACE", "0") == "1"
    if trace:
        trace = _install_ntff_hook()
    try:
        res = run_bass_kernel_spmd(nc, in_maps, core_ids, trace=trace)
    except Exception:
        if not trace:
            raise
        res = run_bass_kernel_spmd(nc, in_maps, core_ids, trace=False)
    LAST_EXEC_NS = res.exec_time_ns
    LAST_RESULTS = res
    out = np.concatenate([res.results[c]["out"] for c in range(NCORES)], axis=0)
    return out.astype(np.float32)


if __name__ == "__main__":
    rng = np.random.default_rng(0)
    a = rng.standard_normal(K).astype(np.float32)
    perm = np.stack([rng.permutation(N) for _ in range(K)]).astype(np.int64)
    T = np.ones((), np.float32)
    out = kernel(a, perm, T)
    # numpy reference
    al = np.exp(a / T - (a / T).max())
    al /= al.sum()
    exp = np.zeros((N, N), np.float32)
    np.add.at(exp, (np.broadcast_to(np.arange(N), (K, N)), perm), al[:, None])
    print("max abs err:", np.abs(out - exp).max(), "max ref:", np.abs(exp).max())
    print("exec ns:", LAST_EXEC_NS)
